# revision 19
# baseline (speedup 1.0000x reference)
"""Switched-FC MoE kernel for Trainium2 (8 NeuronCores, data-parallel).

Math (per token b, expert e = y_index[b]):
    r = relu(x[b])
    h = relu(r @ W1[e] + b1[e])
    o = h @ W2[e] + b2[e]
    out[b] = x[b] + o * z[b]

Strategy:
  * Host: sort tokens by expert; pad each expert's token list to a multiple
    of 8 so every core gets an identical per-expert token count c_e.  This
    makes ONE Bass program (static per-block expert ids baked in at trace
    time) valid for all 8 cores.
  * Host: pre-relu + permute + transpose x so the device reads [D, C] tiles
    with the contraction dim on SBUF partitions (no on-device transpose).
  * Device: weights resident in SBUF; per MBLK-token sub-block (single
    expert): h^T = relu(W1[e]^T @ r^T + b1[e]); o^T = W2[e]^T @ h^T.
    b2 is folded into the host epilogue.
  * Host: unpermute + out = x + z * (o + b2[y]).
  * DMA discipline: per-`dma_start` cost on a HWDGE ring is ~2.5-3 us of
    serialized setup/completion latency in this environment, so the token
    stream is moved in a FEW large transfers: IO_CHUNKS chunks per
    direction, inputs on the SP ring (nc.sync), outputs on the ACT ring
    (nc.scalar) so the two directions don't head-of-line block each other.
  * Activations cross HBM in bf16 (stage-2 mm runs in fp32 from fp32 PSUM
    h).  Set IN_BF16/OUT_BF16 = False for full fp32.
"""

import numpy as np

N_CORES = 8
MBLK = 512      # matmul moving-dim sub-block (fp32 PSUM bank limit)
IO_CHUNKS = 2   # token-stream DMA chunks per direction

IN_BF16 = True   # rin + W1 in bf16 (halves input traffic)
OUT_BF16 = True  # oout in bf16 (halves output traffic)

_PROGRAM_CACHE = {}


def _np_dt(bf16):
    import ml_dtypes
    return ml_dtypes.bfloat16 if bf16 else np.float32


def _chunk_plan(blocks, C, io_chunks):
    """Split [0, C) into io_chunks ranges on the MBLK grid; within each
    chunk list the (expert, start, len) compute pieces (<= MBLK, single
    expert).  `blocks` are the per-expert column segments."""
    n_grid = -(-C // MBLK)
    per = -(-n_grid // io_chunks)
    chunks = []
    for ci in range(io_chunks):
        q0 = min(ci * per * MBLK, C)
        q1 = min((ci + 1) * per * MBLK, C)
        if q0 >= q1:
            continue
        pieces = []
        for (e, t0, n) in blocks:
            lo = max(t0, q0)
            hi = min(t0 + n, q1)
            s = lo
            while s < hi:
                ln = min(MBLK, hi - s)
                pieces.append((e, s, ln))
                s += ln
        chunks.append((q0, q1, pieces))
    return chunks


def _get_program(blocks, C, D, S, E, loop_n=1, in_bf16=IN_BF16,
                 out_bf16=OUT_BF16, io_chunks=IO_CHUNKS,
                 bufs=(2, 3, 2, 2, 3), copy_split=0, stages=5):
    """Build (or fetch cached) compiled Bass program for a block structure.

    loop_n > 1 wraps the body in an on-device For_i so it runs loop_n times
    back-to-back — used only for timing (amortizes dispatch overhead).
    bufs = (xin, h, osb, hps, ops) tile-pool buffer counts.
    copy_split: how many of the 4 per-sub-block PSUM->SBUF copies go to the
    ACT engine instead of DVE (load balancing).
    stages: dev ablation ladder — 1=in-DMA only, 2=+mm1+relu, 3=+mm2,
    4=+copies, 5=full (with out-DMA).
    """
    key = (tuple(blocks), C, D, S, E, loop_n, in_bf16, out_bf16, io_chunks,
           bufs, copy_split, stages)
    if key in _PROGRAM_CACHE:
        return _PROGRAM_CACHE[key]

    import sys
    if "/opt/trn_rl_repo" not in sys.path:
        sys.path.insert(0, "/opt/trn_rl_repo")
    from contextlib import ExitStack

    import concourse.tile as tile
    from concourse import bacc, mybir

    DCH = D // 128  # number of 128-partition chunks of the model dim

    f32 = mybir.dt.float32
    dt_in = mybir.dt.bfloat16 if in_bf16 else f32
    dt_out = mybir.dt.bfloat16 if out_bf16 else f32
    Relu = mybir.ActivationFunctionType.Relu
    nc = bacc.Bacc("TRN2", target_bir_lowering=False, debug=False,
                   num_devices=N_CORES)
    rin = nc.dram_tensor("rin", [D, C], dt_in, kind="ExternalInput").ap()
    w1i = nc.dram_tensor("w1i", [128, DCH * E * S], dt_in,
                         kind="ExternalInput").ap()
    w2i = nc.dram_tensor("w2i", [128, E * D], f32, kind="ExternalInput").ap()
    b1i = nc.dram_tensor("b1i", [128, E], f32, kind="ExternalInput").ap()
    oout = nc.dram_tensor("oout", [D, C], dt_out, kind="ExternalOutput").ap()

    chunks = _chunk_plan(blocks, C, io_chunks)

    with tile.TileContext(nc) as tc, ExitStack() as ctx:
        wpool = ctx.enter_context(tc.tile_pool(name="weights", bufs=1))
        xpool = ctx.enter_context(tc.tile_pool(name="xin", bufs=bufs[0]))
        hpool = ctx.enter_context(tc.tile_pool(name="h", bufs=bufs[1]))
        opool = ctx.enter_context(tc.tile_pool(name="osb", bufs=bufs[2]))
        hps = ctx.enter_context(tc.tile_pool(name="hps", bufs=bufs[3],
                                             space="PSUM"))
        ops = ctx.enter_context(tc.tile_pool(name="ops", bufs=bufs[4],
                                             space="PSUM"))

        # Weights ride the ACT ring (idle during the input phase).
        w1s = wpool.tile([128, DCH * E * S], dt_in)
        nc.scalar.dma_start(w1s[:], w1i)
        w2s = wpool.tile([128, E * D], f32)
        nc.scalar.dma_start(w2s[:], w2i)
        b1s = wpool.tile([128, E], f32)
        nc.scalar.dma_start(b1s[:], b1i)

        def emit_stage1(ci, xt, nq, q0, piece):
            """mm1 accumulation + relu for one piece; returns hs tile."""
            (e, s, ns) = piece
            so = s - q0
            hp = hps.tile([128, ns], f32, tag="hp")
            for c in range(DCH):
                nc.tensor.matmul(
                    hp[:],
                    w1s[:, (e * DCH + c) * S:(e * DCH + c + 1) * S],
                    xt[:, c * nq + so:c * nq + so + ns],
                    start=(c == 0), stop=(c == DCH - 1),
                )
            hs = hpool.tile([128, ns], f32, tag="hs")
            nc.scalar.activation(hs[:], hp[:], Relu, bias=b1s[:, e:e + 1])
            return hs

        def emit_stage2(hs, ot3, q0, piece):
            (e, s, ns) = piece
            so = s - q0
            for m in range(DCH):
                if stages < 3:
                    continue
                op = ops.tile([128, MBLK], f32, tag="op")
                nc.tensor.matmul(
                    op[:, :ns],
                    w2s[:, e * D + m * 128:e * D + (m + 1) * 128],
                    hs[:],
                    start=True, stop=True,
                )
                if stages < 4:
                    continue
                if m >= DCH - copy_split:
                    nc.scalar.activation(
                        ot3[:, m, so:so + ns], op[:, :ns],
                        mybir.ActivationFunctionType.Copy)
                else:
                    nc.vector.tensor_copy(ot3[:, m, so:so + ns],
                                          op[:, :ns])

        def body():
            # One flat work list: (chunk_index, piece).  Software-pipelined
            # emission — stage1 of piece i+1 is emitted BEFORE stage2 of
            # piece i so the in-order PE queue never stalls on ACT/DVE.
            work = []
            xts, ots, ot3s = {}, {}, {}
            last_piece_of_chunk = {}
            for ci, (q0, q1, pieces) in enumerate(chunks):
                for pi, piece in enumerate(pieces):
                    work.append((ci, piece))
                last_piece_of_chunk[ci] = len(work) - 1

            def ensure_chunk(ci):
                q0, q1, _ = chunks[ci]
                nq = q1 - q0
                if ci not in xts:
                    xt = xpool.tile([128, DCH * nq], dt_in, tag="xt",
                                    name=f"xt{ci % max(bufs[0], 1)}")
                    src = rin[:, q0:q1].rearrange("(c p) t -> p c t", p=128)
                    dst = xt[:].rearrange("p (c t) -> p c t", c=DCH)
                    nc.sync.dma_start(dst, src)
                    xts[ci] = xt
                    ot = opool.tile([128, DCH * nq], dt_out, tag="ot",
                                    name=f"ot{ci % max(bufs[2], 1)}")
                    ots[ci] = ot
                    ot3s[ci] = ot[:].rearrange("p (m t) -> p m t", m=DCH)

            def flush_chunk(ci):
                if stages >= 5:
                    q0, q1, _ = chunks[ci]
                    osrc = ots[ci][:].rearrange("p (c t) -> p c t", c=DCH)
                    odst = oout[:, q0:q1].rearrange("(c p) t -> p c t", p=128)
                    nc.scalar.dma_start(odst, osrc)

            if stages < 2:
                for ci in range(len(chunks)):
                    ensure_chunk(ci)
                return

            pending = None  # (ci, piece, hs) awaiting stage2
            for wi, (ci, piece) in enumerate(work):
                ensure_chunk(ci)
                q0 = chunks[ci][0]
                hs = emit_stage1(ci, xts[ci], chunks[ci][1] - q0, q0, piece)
                if pending is not None:
                    (pci, ppiece, phs) = pending
                    emit_stage2(phs, ot3s[pci], chunks[pci][0], ppiece)
                    if last_piece_of_chunk[pci] == wi - 1:
                        flush_chunk(pci)
                pending = (ci, piece, hs)
            if pending is not None:
                (pci, ppiece, phs) = pending
                emit_stage2(phs, ot3s[pci], chunks[pci][0], ppiece)
                flush_chunk(pci)

        if loop_n == 1:
            body()
        else:
            with tc.For_i(0, loop_n, 1):
                body()

    nc.compile()
    _PROGRAM_CACHE[key] = nc
    return nc


def _plan(yi, E):
    """Token permutation plan: per-core per-expert counts identical across
    cores, so one program serves all cores."""
    order = np.argsort(yi, kind="stable")
    counts = np.bincount(yi, minlength=E)
    c = -(-counts // N_CORES)  # ceil
    C = int(c.sum())
    perm = np.zeros((N_CORES, C), dtype=np.int64)
    valid = np.zeros((N_CORES, C), dtype=bool)
    blocks = []
    off = 0
    col = 0
    for e in range(E):
        n_e = int(counts[e])
        ce = int(c[e])
        if ce == 0:
            continue
        seg = order[off:off + n_e]
        padded = np.empty(N_CORES * ce, dtype=np.int64)
        padded[:n_e] = seg
        padded[n_e:] = seg[-1] if n_e > 0 else 0
        v = np.zeros(N_CORES * ce, dtype=bool)
        v[:n_e] = True
        perm[:, col:col + ce] = padded.reshape(N_CORES, ce)
        valid[:, col:col + ce] = v.reshape(N_CORES, ce)
        blocks.append((e, col, ce))
        off += n_e
        col += ce
    assert col == C
    return blocks, perm, valid, C


def _prep_inputs(x, yi, z, W1, b1, W2, b2, in_bf16=IN_BF16):
    """Host-side routing + layout prep shared by kernel() and the timing
    harness.  Returns (blocks, perm, valid, C, in_maps)."""
    B, D = x.shape
    E, _, S = W1.shape
    DCH = D // 128
    dt_in = _np_dt(in_bf16)

    blocks, perm, valid, C = _plan(yi, E)

    r = np.maximum(x, 0.0)
    rin = np.ascontiguousarray(
        r[perm.reshape(-1)].reshape(N_CORES, C, D).transpose(0, 2, 1)
    ).astype(dt_in)

    w1i = np.ascontiguousarray(
        W1.reshape(E, DCH, 128, S).transpose(2, 0, 1, 3)
        .reshape(128, E * DCH * S)).astype(dt_in)
    w2i = np.ascontiguousarray(W2.transpose(1, 0, 2).reshape(128, E * D))
    b1i = np.ascontiguousarray(b1.T)  # [S=128, E]

    in_maps = [
        {"rin": rin[m], "w1i": w1i, "w2i": w2i, "b1i": b1i}
        for m in range(N_CORES)
    ]
    return blocks, perm, valid, C, in_maps


def kernel(x, y_index, y_hard, z, W1, b1, W2, b2):
    import sys
    if "/opt/trn_rl_repo" not in sys.path:
        sys.path.insert(0, "/opt/trn_rl_repo")
    from concourse import bass_utils

    x = np.ascontiguousarray(np.asarray(x, dtype=np.float32))
    z = np.asarray(z, dtype=np.float32)
    W1 = np.asarray(W1, dtype=np.float32)
    b1 = np.asarray(b1, dtype=np.float32)
    W2 = np.asarray(W2, dtype=np.float32)
    b2 = np.asarray(b2, dtype=np.float32)
    yi = np.asarray(y_index).reshape(-1).astype(np.int64)

    B, D = x.shape
    E, _, S = W1.shape

    blocks, perm, valid, C, in_maps = _prep_inputs(x, yi, z, W1, b1, W2, b2)
    nc = _get_program(blocks, C, D, S, E)

    res = bass_utils.run_bass_kernel_spmd(nc, in_maps,
                                          core_ids=list(range(N_CORES)))

    # Gather: oout[m] is [D, C]; o for padded slot (m, t) lives at [:, t].
    o_perm = np.stack(
        [np.asarray(res.results[m]["oout"], dtype=np.float32)
         for m in range(N_CORES)], axis=0)
    o_perm = o_perm.transpose(0, 2, 1).reshape(N_CORES * C, D)

    vflat = valid.reshape(-1)
    dest = perm.reshape(-1)[vflat]
    out = x.copy()
    out[dest] = x[dest] + z[dest] * (o_perm[vflat] + b2[yi[dest]])
    return out


# revision 20
# speedup vs baseline: 1.6680x; 1.6680x over previous
"""Switched-FC MoE kernel for Trainium2 (8 NeuronCores, data-parallel).

Math (per token b, expert e = y_index[b]):
    r = relu(x[b])
    h = relu(r @ W1[e] + b1[e])
    o = h @ W2[e] + b2[e]
    out[b] = x[b] + o * z[b]

Strategy:
  * Host: sort tokens by expert; pad each expert's token list to a multiple
    of 8 so every core gets an identical per-expert token count c_e.  This
    makes ONE Bass program (static per-block expert ids baked in at trace
    time) valid for all 8 cores.
  * Host: pre-relu + permute + transpose x so the device reads [D, C] tiles
    with the contraction dim on SBUF partitions (no on-device transpose).
  * Device: weights resident in SBUF; per MBLK-token sub-block (single
    expert): h^T = relu(W1[e]^T @ r^T + b1[e]); o^T = W2[e]^T @ h^T.
    b2 is folded into the host epilogue.
  * Host: unpermute + out = x + z * (o + b2[y]).
  * DMA discipline: per-`dma_start` cost on a HWDGE ring is ~2.5-3 us of
    serialized setup/completion latency in this environment, so the token
    stream is moved in a FEW large transfers: IO_CHUNKS chunks per
    direction, inputs on the SP ring (nc.sync), outputs on the ACT ring
    (nc.scalar) so the two directions don't head-of-line block each other.
  * All matmuls run in bf16 (fp32 matmul streams ~3.4x slower on this
    part); PSUM accumulation stays fp32.  Set IN_BF16/OUT_BF16 = False for
    full fp32.
"""

import numpy as np

N_CORES = 8
MBLK = 512      # matmul moving-dim sub-block (fp32 PSUM bank limit)
IO_CHUNKS = 2   # token-stream DMA chunks per direction

IN_BF16 = True   # rin + W1 in bf16 (halves input traffic)
OUT_BF16 = True  # oout in bf16 (halves output traffic)

_PROGRAM_CACHE = {}


def _np_dt(bf16):
    import ml_dtypes
    return ml_dtypes.bfloat16 if bf16 else np.float32


def _chunk_plan(blocks, C, io_chunks):
    """Split [0, C) into io_chunks ranges on the MBLK grid; within each
    chunk list the (expert, start, len) compute pieces (<= MBLK, single
    expert).  `blocks` are the per-expert column segments."""
    n_grid = -(-C // MBLK)
    per = -(-n_grid // io_chunks)
    chunks = []
    for ci in range(io_chunks):
        q0 = min(ci * per * MBLK, C)
        q1 = min((ci + 1) * per * MBLK, C)
        if q0 >= q1:
            continue
        pieces = []
        for (e, t0, n) in blocks:
            lo = max(t0, q0)
            hi = min(t0 + n, q1)
            s = lo
            while s < hi:
                ln = min(MBLK, hi - s)
                pieces.append((e, s, ln))
                s += ln
        chunks.append((q0, q1, pieces))
    return chunks


def _get_program(blocks, C, D, S, E, loop_n=1, in_bf16=IN_BF16,
                 out_bf16=OUT_BF16, io_chunks=IO_CHUNKS,
                 bufs=(2, 3, 2, 2, 3), copy_split=0, stages=5):
    """Build (or fetch cached) compiled Bass program for a block structure.

    loop_n > 1 wraps the body in an on-device For_i so it runs loop_n times
    back-to-back — used only for timing (amortizes dispatch overhead).
    bufs = (xin, h, osb, hps, ops) tile-pool buffer counts.
    copy_split: how many of the 4 per-sub-block PSUM->SBUF copies go to the
    ACT engine instead of DVE (load balancing).
    stages: dev ablation ladder — 1=in-DMA only, 2=+mm1+relu, 3=+mm2,
    4=+copies, 5=full (with out-DMA).
    """
    key = (tuple(blocks), C, D, S, E, loop_n, in_bf16, out_bf16, io_chunks,
           bufs, copy_split, stages)
    if key in _PROGRAM_CACHE:
        return _PROGRAM_CACHE[key]

    import sys
    if "/opt/trn_rl_repo" not in sys.path:
        sys.path.insert(0, "/opt/trn_rl_repo")
    from contextlib import ExitStack

    import concourse.tile as tile
    from concourse import bacc, mybir

    DCH = D // 128  # number of 128-partition chunks of the model dim

    f32 = mybir.dt.float32
    dt_in = mybir.dt.bfloat16 if in_bf16 else f32
    dt_out = mybir.dt.bfloat16 if out_bf16 else f32
    Relu = mybir.ActivationFunctionType.Relu
    nc = bacc.Bacc("TRN2", target_bir_lowering=False, debug=False,
                   num_devices=N_CORES)
    rin = nc.dram_tensor("rin", [D, C], dt_in, kind="ExternalInput").ap()
    w1i = nc.dram_tensor("w1i", [128, DCH * E * S], dt_in,
                         kind="ExternalInput").ap()
    w2i = nc.dram_tensor("w2i", [128, E * D], dt_in,
                         kind="ExternalInput").ap()
    b1i = nc.dram_tensor("b1i", [128, E], f32, kind="ExternalInput").ap()
    oout = nc.dram_tensor("oout", [D, C], dt_out, kind="ExternalOutput").ap()

    chunks = _chunk_plan(blocks, C, io_chunks)

    with tile.TileContext(nc) as tc, ExitStack() as ctx:
        wpool = ctx.enter_context(tc.tile_pool(name="weights", bufs=1))
        xpool = ctx.enter_context(tc.tile_pool(name="xin", bufs=bufs[0]))
        hpool = ctx.enter_context(tc.tile_pool(name="h", bufs=bufs[1]))
        opool = ctx.enter_context(tc.tile_pool(name="osb", bufs=bufs[2]))
        hps = ctx.enter_context(tc.tile_pool(name="hps", bufs=bufs[3],
                                             space="PSUM"))
        ops = ctx.enter_context(tc.tile_pool(name="ops", bufs=bufs[4],
                                             space="PSUM"))

        # Weights ride the ACT ring (idle during the input phase).
        w1s = wpool.tile([128, DCH * E * S], dt_in)
        nc.scalar.dma_start(w1s[:], w1i)
        w2s = wpool.tile([128, E * D], dt_in)
        nc.scalar.dma_start(w2s[:], w2i)
        b1s = wpool.tile([128, E], f32)
        nc.scalar.dma_start(b1s[:], b1i)

        def emit_stage1(ci, xt, nq, q0, piece):
            """mm1 accumulation + relu for one piece; returns hs tile."""
            (e, s, ns) = piece
            so = s - q0
            hp = hps.tile([128, ns], f32, tag="hp")
            for c in range(DCH):
                nc.tensor.matmul(
                    hp[:],
                    w1s[:, (e * DCH + c) * S:(e * DCH + c + 1) * S],
                    xt[:, c * nq + so:c * nq + so + ns],
                    start=(c == 0), stop=(c == DCH - 1),
                )
            hs = hpool.tile([128, ns], dt_in, tag="hs")
            nc.scalar.activation(hs[:], hp[:], Relu, bias=b1s[:, e:e + 1])
            return hs

        def emit_stage2(hs, ot3, q0, piece):
            (e, s, ns) = piece
            so = s - q0
            for m in range(DCH):
                if stages < 3:
                    continue
                op = ops.tile([128, MBLK], f32, tag="op")
                nc.tensor.matmul(
                    op[:, :ns],
                    w2s[:, e * D + m * 128:e * D + (m + 1) * 128],
                    hs[:],
                    start=True, stop=True,
                )
                if stages < 4:
                    continue
                if m >= DCH - copy_split:
                    nc.scalar.activation(
                        ot3[:, m, so:so + ns], op[:, :ns],
                        mybir.ActivationFunctionType.Copy)
                else:
                    nc.vector.tensor_copy(ot3[:, m, so:so + ns],
                                          op[:, :ns])

        def body():
            # One flat work list: (chunk_index, piece).  Software-pipelined
            # emission — stage1 of piece i+1 is emitted BEFORE stage2 of
            # piece i so the in-order PE queue never stalls on ACT/DVE.
            work = []
            xts, ots, ot3s = {}, {}, {}
            last_piece_of_chunk = {}
            for ci, (q0, q1, pieces) in enumerate(chunks):
                for pi, piece in enumerate(pieces):
                    work.append((ci, piece))
                last_piece_of_chunk[ci] = len(work) - 1

            def ensure_chunk(ci):
                q0, q1, _ = chunks[ci]
                nq = q1 - q0
                if ci not in xts:
                    xt = xpool.tile([128, DCH * nq], dt_in, tag="xt",
                                    name=f"xt{ci % max(bufs[0], 1)}")
                    src = rin[:, q0:q1].rearrange("(c p) t -> p c t", p=128)
                    dst = xt[:].rearrange("p (c t) -> p c t", c=DCH)
                    nc.sync.dma_start(dst, src)
                    xts[ci] = xt
                    ot = opool.tile([128, DCH * nq], dt_out, tag="ot",
                                    name=f"ot{ci % max(bufs[2], 1)}")
                    ots[ci] = ot
                    ot3s[ci] = ot[:].rearrange("p (m t) -> p m t", m=DCH)

            def flush_chunk(ci):
                if stages >= 5:
                    q0, q1, _ = chunks[ci]
                    osrc = ots[ci][:].rearrange("p (c t) -> p c t", c=DCH)
                    odst = oout[:, q0:q1].rearrange("(c p) t -> p c t", p=128)
                    nc.scalar.dma_start(odst, osrc)

            if stages < 2:
                for ci in range(len(chunks)):
                    ensure_chunk(ci)
                return

            pending = None  # (ci, piece, hs) awaiting stage2
            for wi, (ci, piece) in enumerate(work):
                ensure_chunk(ci)
                q0 = chunks[ci][0]
                hs = emit_stage1(ci, xts[ci], chunks[ci][1] - q0, q0, piece)
                if pending is not None:
                    (pci, ppiece, phs) = pending
                    emit_stage2(phs, ot3s[pci], chunks[pci][0], ppiece)
                    if last_piece_of_chunk[pci] == wi - 1:
                        flush_chunk(pci)
                pending = (ci, piece, hs)
            if pending is not None:
                (pci, ppiece, phs) = pending
                emit_stage2(phs, ot3s[pci], chunks[pci][0], ppiece)
                flush_chunk(pci)

        if loop_n == 1:
            body()
        else:
            with tc.For_i(0, loop_n, 1):
                body()

    nc.compile()
    _PROGRAM_CACHE[key] = nc
    return nc


def _plan(yi, E):
    """Token permutation plan: per-core per-expert counts identical across
    cores, so one program serves all cores."""
    order = np.argsort(yi, kind="stable")
    counts = np.bincount(yi, minlength=E)
    c = -(-counts // N_CORES)  # ceil
    C = int(c.sum())
    perm = np.zeros((N_CORES, C), dtype=np.int64)
    valid = np.zeros((N_CORES, C), dtype=bool)
    blocks = []
    off = 0
    col = 0
    for e in range(E):
        n_e = int(counts[e])
        ce = int(c[e])
        if ce == 0:
            continue
        seg = order[off:off + n_e]
        padded = np.empty(N_CORES * ce, dtype=np.int64)
        padded[:n_e] = seg
        padded[n_e:] = seg[-1] if n_e > 0 else 0
        v = np.zeros(N_CORES * ce, dtype=bool)
        v[:n_e] = True
        perm[:, col:col + ce] = padded.reshape(N_CORES, ce)
        valid[:, col:col + ce] = v.reshape(N_CORES, ce)
        blocks.append((e, col, ce))
        off += n_e
        col += ce
    assert col == C
    return blocks, perm, valid, C


def _prep_inputs(x, yi, z, W1, b1, W2, b2, in_bf16=IN_BF16):
    """Host-side routing + layout prep shared by kernel() and the timing
    harness.  Returns (blocks, perm, valid, C, in_maps)."""
    B, D = x.shape
    E, _, S = W1.shape
    DCH = D // 128
    dt_in = _np_dt(in_bf16)

    blocks, perm, valid, C = _plan(yi, E)

    r = np.maximum(x, 0.0)
    rin = np.ascontiguousarray(
        r[perm.reshape(-1)].reshape(N_CORES, C, D).transpose(0, 2, 1)
    ).astype(dt_in)

    w1i = np.ascontiguousarray(
        W1.reshape(E, DCH, 128, S).transpose(2, 0, 1, 3)
        .reshape(128, E * DCH * S)).astype(dt_in)
    w2i = np.ascontiguousarray(
        W2.transpose(1, 0, 2).reshape(128, E * D)).astype(dt_in)
    b1i = np.ascontiguousarray(b1.T)  # [S=128, E]

    in_maps = [
        {"rin": rin[m], "w1i": w1i, "w2i": w2i, "b1i": b1i}
        for m in range(N_CORES)
    ]
    return blocks, perm, valid, C, in_maps


def kernel(x, y_index, y_hard, z, W1, b1, W2, b2):
    import sys
    if "/opt/trn_rl_repo" not in sys.path:
        sys.path.insert(0, "/opt/trn_rl_repo")
    from concourse import bass_utils

    x = np.ascontiguousarray(np.asarray(x, dtype=np.float32))
    z = np.asarray(z, dtype=np.float32)
    W1 = np.asarray(W1, dtype=np.float32)
    b1 = np.asarray(b1, dtype=np.float32)
    W2 = np.asarray(W2, dtype=np.float32)
    b2 = np.asarray(b2, dtype=np.float32)
    yi = np.asarray(y_index).reshape(-1).astype(np.int64)

    B, D = x.shape
    E, _, S = W1.shape

    blocks, perm, valid, C, in_maps = _prep_inputs(x, yi, z, W1, b1, W2, b2)
    nc = _get_program(blocks, C, D, S, E)

    res = bass_utils.run_bass_kernel_spmd(nc, in_maps,
                                          core_ids=list(range(N_CORES)))

    # Gather: oout[m] is [D, C]; o for padded slot (m, t) lives at [:, t].
    o_perm = np.stack(
        [np.asarray(res.results[m]["oout"], dtype=np.float32)
         for m in range(N_CORES)], axis=0)
    o_perm = o_perm.transpose(0, 2, 1).reshape(N_CORES * C, D)

    vflat = valid.reshape(-1)
    dest = perm.reshape(-1)[vflat]
    out = x.copy()
    out[dest] = x[dest] + z[dest] * (o_perm[vflat] + b2[yi[dest]])
    return out
